# revision 1
# baseline (speedup 1.0000x reference)
"""u_dot_v edge scoring on 8 Trainium2 NeuronCores.

score[e] = dot(h[src[e]], h[dst[e]])  for 600k edges, 128-dim features.

Strategy (edge-parallel, memory-bound):
  - Shard edges across 8 cores (75k each); replicate h in each core's DRAM.
  - Node ids don't fit the int16 index type of the bulk-gather DMA
    (dma_gather), so nodes are split into ranges of 32768 and each core's
    edges are bucketed host-side by (src_range, dst_range). Within a bucket
    both endpoints are range-local (< 32768) and hu/hv gathers stay
    edge-aligned.
  - Per bucket: two dma_gather ops (512B row per edge endpoint, descriptors
    generated by Q7 SWDGE, drained by 16 SDMA engines), an elementwise
    multiply, and a segmented reduce over the feature axis into a resident
    score buffer, written back once at the end.
  - Bucket capacities are the max count over cores (rounded to 128) so the
    same program serves all cores SPMD; padding slots gather row 0 of the
    range and are dropped host-side. Scores come back bucket-ordered; the
    host inverse-permutes.
"""

import numpy as np

from concourse import bacc, mybir, tile
from concourse.bass_utils import run_bass_kernel_spmd

P = 128
N_NODES = 100000
D_FEAT = 128
N_EDGES = 600000
N_CORES = 8
RANGE_BITS = 15
RANGE_LEN = 1 << RANGE_BITS
MAX_CHUNK_COLS = 8  # max edge-columns per dma_gather (SWDGE ring: 1024 descs)


def _make_plan(n_nodes, d, cap):
    """Static per-bucket layout shared by all cores.

    cap[b]: padded edge count of bucket b (multiple of 128).
    Returns list of dicts with DRAM/SBUF offsets per gather chunk.
    """
    n_ranges = (n_nodes + RANGE_LEN - 1) >> RANGE_BITS
    plan = []
    woff = 0  # idx sbuf column offset (int16, 16 tokens/col)
    toff = 0  # score/output column offset (128 edges/col)
    for b in range(n_ranges * n_ranges):
        rs, rd = divmod(b, n_ranges)
        src_base = rs << RANGE_BITS
        dst_base = rd << RANGE_BITS
        chunks = []
        done = 0
        while done < cap[b]:
            n = min(cap[b] - done, MAX_CHUNK_COLS * P)
            chunks.append({
                "n_idx": n,
                "woff": woff,
                "toff": toff,
                "cols": n // P,
            })
            woff += n // 16
            toff += n // P
            done += n
        plan.append({
            "bucket": b,
            "src_base": src_base,
            "src_len": min(RANGE_LEN, n_nodes - src_base),
            "dst_base": dst_base,
            "dst_len": min(RANGE_LEN, n_nodes - dst_base),
            "chunks": chunks,
        })
    return plan, woff, toff


def emit_body(tcx, outs, ins, plan, w_total, t_total):
    nc = tcx.nc
    h = ins["h"]
    src_d = ins["src_idx"]
    dst_d = ins["dst_idx"]
    out = outs["score"]
    d = h.shape[1]

    max_cols = max(c["cols"] for p in plan for c in p["chunks"])
    n_queues = nc.num_swdge_queues
    qn = 0

    with tcx.tile_pool(name="idx", bufs=1) as idx_pool, \
         tcx.tile_pool(name="gath", bufs=8) as gpool, \
         tcx.tile_pool(name="sc", bufs=1) as spool:
        src_sb = idx_pool.tile([P, w_total], mybir.dt.int16, tag="src")
        dst_sb = idx_pool.tile([P, w_total], mybir.dt.int16, tag="dst")
        score_sb = spool.tile([P, t_total], mybir.dt.float32)
        nc.sync.dma_start(out=src_sb[:], in_=src_d[:, :])
        nc.sync.dma_start(out=dst_sb[:], in_=dst_d[:, :])

        for pb in plan:
            hs = h[pb["src_base"]:pb["src_base"] + pb["src_len"], :]
            hd = h[pb["dst_base"]:pb["dst_base"] + pb["dst_len"], :]
            for ch in pb["chunks"]:
                n_idx, woff, toff, cols = (
                    ch["n_idx"], ch["woff"], ch["toff"], ch["cols"])
                hu = gpool.tile([P, max_cols, d], mybir.dt.float32, tag="hu")
                hv = gpool.tile([P, max_cols, d], mybir.dt.float32, tag="hv")
                nc.gpsimd.dma_gather(
                    hu[:, :cols, :], hs, src_sb[:, woff:woff + n_idx // 16],
                    n_idx, n_idx, d, queue_num=qn % n_queues)
                qn += 1
                nc.gpsimd.dma_gather(
                    hv[:, :cols, :], hd, dst_sb[:, woff:woff + n_idx // 16],
                    n_idx, n_idx, d, queue_num=qn % n_queues)
                qn += 1
                nc.vector.tensor_tensor(
                    out=hu[:, :cols, :], in0=hu[:, :cols, :],
                    in1=hv[:, :cols, :], op=mybir.AluOpType.mult)
                nc.vector.tensor_reduce(
                    out=score_sb[:, toff:toff + cols], in_=hu[:, :cols, :],
                    axis=mybir.AxisListType.X, op=mybir.AluOpType.add)

        nc.sync.dma_start(out=out[:, :], in_=score_sb[:])


def _build(n_nodes, d, cap):
    plan, w_total, t_total = _make_plan(n_nodes, d, cap)
    nc = bacc.Bacc("TRN2", target_bir_lowering=False, debug=False,
                   enable_asserts=False, num_swdge_queues=4)
    h = nc.dram_tensor("h", [n_nodes, d], mybir.dt.float32,
                       kind="ExternalInput").ap()
    src = nc.dram_tensor("src_idx", [P, w_total], mybir.dt.int16,
                         kind="ExternalInput").ap()
    dst = nc.dram_tensor("dst_idx", [P, w_total], mybir.dt.int16,
                         kind="ExternalInput").ap()
    out = nc.dram_tensor("score", [P, t_total], mybir.dt.float32,
                         kind="ExternalOutput").ap()
    with tile.TileContext(nc) as tcx:
        emit_body(tcx, {"score": out},
                  {"h": h, "src_idx": src, "dst_idx": dst},
                  plan, w_total, t_total)
    nc.compile()
    return nc, plan, w_total, t_total


def _bucketize(src_c, dst_c, n_ranges):
    bucket = (src_c >> RANGE_BITS) * n_ranges + (dst_c >> RANGE_BITS)
    perm = np.argsort(bucket, kind="stable")
    counts = np.bincount(bucket, minlength=n_ranges * n_ranges)
    return bucket, perm, counts


def _wrap_idx16(vals, cap):
    """Pad local indices to cap with 0, lay out 16-wrapped + replicated.

    Token k lives at [k % 16 + 16*g, k // 16] for every group g.
    Returns [P, cap // 16] int16.
    """
    buf = np.zeros(cap, np.int16)
    buf[:vals.shape[0]] = vals.astype(np.int16)
    w = buf.reshape(cap // 16, 16).T  # [16, cap//16]
    return np.tile(w, (P // 16, 1))


def _run(h, src, dst, trace=False, **run_kwargs):
    h32 = np.ascontiguousarray(np.asarray(h, dtype=np.float32))
    src = np.asarray(src).astype(np.int64)
    dst = np.asarray(dst).astype(np.int64)
    n_nodes, d = h32.shape
    e = src.shape[0]
    e_core = e // N_CORES
    assert e_core * N_CORES == e
    n_ranges = (n_nodes + RANGE_LEN - 1) >> RANGE_BITS

    # Per-core bucketing, shared static capacities
    per_core = []
    for c in range(N_CORES):
        sl = slice(c * e_core, (c + 1) * e_core)
        per_core.append(_bucketize(src[sl], dst[sl], n_ranges))
    max_counts = np.max([pc[2] for pc in per_core], axis=0)
    cap = np.maximum(((max_counts + P - 1) // P) * P, P).astype(np.int64)

    nc, plan, w_total, t_total = _build(n_nodes, d, cap)

    in_maps = []
    for c in range(N_CORES):
        sl = slice(c * e_core, (c + 1) * e_core)
        src_c, dst_c = src[sl], dst[sl]
        _, perm, counts = per_core[c]
        src16 = np.zeros((P, w_total), np.int16)
        dst16 = np.zeros((P, w_total), np.int16)
        off = 0
        for pb in plan:
            b = pb["bucket"]
            n = int(counts[b])
            es = perm[off:off + n]
            sv = src_c[es] - pb["src_base"]
            dv = dst_c[es] - pb["dst_base"]
            cb = int(cap[b])
            w0 = pb["chunks"][0]["woff"]
            src16[:, w0:w0 + cb // 16] = _wrap_idx16(sv, cb)
            dst16[:, w0:w0 + cb // 16] = _wrap_idx16(dv, cb)
            off += n
        in_maps.append({"h": h32, "src_idx": src16, "dst_idx": dst16})

    res = run_bass_kernel_spmd(nc, in_maps, core_ids=list(range(N_CORES)),
                               trace=trace, **run_kwargs)

    out = np.empty((e, 1), np.float32)
    for c in range(N_CORES):
        sc = res.results[c]["score"]  # [P, t_total]
        _, perm, counts = per_core[c]
        out_c = np.empty(e_core, np.float32)
        off = 0
        for pb in plan:
            b = pb["bucket"]
            n = int(counts[b])
            t0 = pb["chunks"][0]["toff"]
            cb = int(cap[b])
            vals = sc[:, t0:t0 + cb // P].T.reshape(-1)[:n]
            out_c[perm[off:off + n]] = vals
            off += n
        out[c * e_core:(c + 1) * e_core, 0] = out_c
    return out, res


def kernel(h, src, dst):
    out, _ = _run(h, src, dst)
    return out

